# revision 41
# baseline (speedup 1.0000x reference)
"""Causal self-attention (B=2, T=2048, D=1024, H=16, dh=64) on 8 TRN2
NeuronCores.

Sharding: 2-way batch parallel x 4-way head (tensor) parallel.
Core c: batch b = c // 4, heads 4*(c%4) .. 4*(c%4)+3.

Per-core program (Megatron-style TP, bf16 matmuls, f32 softmax stats):
  QT = Wq_s.T @ x_b.T          [256, 2048]  (head dims on partitions)
  KT = Wk_s.T @ x_b.T          [256, 2048]
  V  = x_b @ Wv_s              [2048, 256]  (stored interleaved with ones col)
  per head h, key tile jt, 512-wide query chunk c:
      ST = Kh_jt @ Qh_c^T (causal-masked on the diagonal chunk)
      PT = exp(ST/8)           (no max subtraction; logits are O(6))
      [OT; L] = [Vh | 1]^T @ PT   accumulated over key tiles in PSUM
      OTn = OT * (1/L broadcast along partitions)
  Ypart = OTn.T @ Wo_s         [2048, 1024] partial over heads
  ReduceScatter(add) over the 4 cores of the batch group, split into two
  query-halves so the first RS overlaps the second half's compute.

Program order interleaves the second head-pair's QKV projections with the
first pair's attention so ScalarE (exp) starts early and the PE stream
stays dense (HAM warm).
"""

import numpy as np

import concourse.bass as bass
import concourse.mybir as mybir
import concourse.tile as tile
from concourse import bacc
from concourse.bass_utils import run_bass_kernel_spmd

P = 128          # partitions
T = 2048         # tokens
D = 1024         # d_model
H_LOC = 4        # heads per core
DH = 64          # head dim
DHC = H_LOC * DH  # 256 head-dim cols per core
KD = D // P      # 8 k-tiles over d_model
NT = T // P      # 16 token tiles
IC = 512         # query chunk width
NIC = T // IC    # 4
NJT = T // P     # 16 key tiles
HW = T // 2      # 1024 query cols per half
SCALE = 1.0 / np.sqrt(DH)
MASKVAL = -1e9

f32 = mybir.dt.float32
bf16 = mybir.dt.bfloat16
f16 = mybir.dt.float16

N_CORES = 8
GROUPS = [[0, 1, 2, 3], [4, 5, 6, 7]]


def build_nc(dbg=False):
    nc = bacc.Bacc("TRN2", target_bir_lowering=False, debug=False,
                   num_devices=N_CORES)

    xT_d = nc.dram_tensor("xT", [D, T], bf16, kind="ExternalInput")
    wq_d = nc.dram_tensor("wq", [D, DHC], bf16, kind="ExternalInput")
    wk_d = nc.dram_tensor("wk", [D, DHC], bf16, kind="ExternalInput")
    wv_d = nc.dram_tensor("wv", [D, DHC], bf16, kind="ExternalInput")
    wo_d = nc.dram_tensor("wo", [DHC, D], bf16, kind="ExternalInput")
    cm_d = nc.dram_tensor("cmask", [P, 4 * IC], f32, kind="ExternalInput")
    ones_d = nc.dram_tensor("ones", [P, H_LOC], bf16, kind="ExternalInput")
    out_d = nc.dram_tensor("out", [T // 4, D], f16, kind="ExternalOutput")
    if dbg:
        dqt = nc.dram_tensor("dbg_qt", [2 * P, T], bf16, kind="ExternalOutput")
        dkt = nc.dram_tensor("dbg_kt", [2 * P, T], bf16, kind="ExternalOutput")
        dva = nc.dram_tensor("dbg_va", [P, H_LOC * (DH + 1)], bf16,
                             kind="ExternalOutput")
        dot = nc.dram_tensor("dbg_otn", [2 * P, T], bf16, kind="ExternalOutput")
        dbc = nc.dram_tensor("dbg_bc", [H_LOC * DH, IC], f32,
                             kind="ExternalOutput")
        dyb = nc.dram_tensor("dbg_yb", [T, D], f32, kind="ExternalOutput")

    with tile.TileContext(nc) as tc:
        with (
            tc.tile_pool(name="persist", bufs=1) as persist,
            tc.tile_pool(name="work", bufs=1) as work,
            tc.tile_pool(name="psum", bufs=1, space="PSUM") as psum,
            tc.tile_pool(name="dram", bufs=1, space="DRAM") as dram,
        ):
            # ---- persistent SBUF tensors ----
            wo_t = []
            for m in range(2):
                w1 = persist.tile([P, D], bf16, name=f"wot{m}")
                nc.sync.dma_start(out=w1[:], in_=wo_d[m * P:(m + 1) * P, :])
                wo_t.append(w1)
            cmask = persist.tile([P, 4 * IC], f32)
            nc.sync.dma_start(out=cmask[:], in_=cm_d[:])
            qt, kt = [], []
            for m in range(2):
                qt.append(persist.tile([P, T], bf16, name=f"qt{m}"))
                kt.append(persist.tile([P, T], bf16, name=f"kt{m}"))
            va = [persist.tile([P, H_LOC * (DH + 1)], bf16, name=f"va{tt}")
                  for tt in range(NT)]
            otn = [persist.tile([P, T], bf16, name=f"otn{m}") for m in range(2)]

            ybounce = dram.tile([T, D], f16, name="ybounce")
            rs_out = [dram.tile([T // 16, D], f16, name=f"rs_out{q}")
                      for q in range(4)]

            # ---- emission helpers ----
            def qtkt_unit(m, ic, wq_t, wk_t, xt):
                def go():
                    psQ = psum.tile([P, IC], f32, tag="ps512", name="psQ",
                                    bufs=4)
                    psK = psum.tile([P, IC], f32, tag="ps512", name="psK",
                                    bufs=4)
                    for k in range(KD):
                        nc.tensor.matmul(
                            psQ[:], wq_t[k][:, m * P:(m + 1) * P],
                            xt[k][:, ic * IC:(ic + 1) * IC],
                            start=(k == 0), stop=(k == KD - 1))
                        nc.tensor.matmul(
                            psK[:], wk_t[k][:, m * P:(m + 1) * P],
                            xt[k][:, ic * IC:(ic + 1) * IC],
                            start=(k == 0), stop=(k == KD - 1))
                    nc.vector.tensor_copy(qt[m][:, ic * IC:(ic + 1) * IC],
                                          psQ[:])
                    nc.vector.tensor_copy(kt[m][:, ic * IC:(ic + 1) * IC],
                                          psK[:])
                return go

            def v_unit(tt, wv_t, xt):
                def go():
                    va3 = va[tt].rearrange("p (h c) -> p h c", c=DH + 1)
                    nc.sync.dma_start(out=va3[:, :, DH:DH + 1],
                                      in_=ones_d[:].unsqueeze(-1))
                    psV = psum.tile([P, DHC], f32, tag="ps512", name="psV",
                                    bufs=4)
                    for k in range(KD):
                        nc.tensor.matmul(
                            psV[:], xt[k][:, tt * P:(tt + 1) * P], wv_t[k][:],
                            start=(k == 0), stop=(k == KD - 1))
                    psV3 = psV.rearrange("p (h c) -> p h c", c=DH)
                    nc.vector.tensor_copy(va3[:, :, 0:DH], psV3[:, :, :])
                return go

            def emit_normalize(h, half, cl, psO):
                # normalize: OTn[d, i] = OT[d, i] * (1/L[i])
                m = h // 2
                even = (h % 2 == 0)
                c = 2 * half + cl
                rl = work.tile([DH + 1, IC], f32, tag="rl", name="rl", bufs=3)
                l0 = work.tile([1, IC], f32, tag="l0", name="l0", bufs=3)
                bc = work.tile([DH, IC], f32, tag="bc", name="bc", bufs=3)
                # Copy the L row out of PSUM at its aligned base, scatter it
                # across 128 partitions so the (multi-op) reciprocal runs on
                # free-size 4 instead of 512, gather the result back to
                # partition 0, and broadcast.
                nc.vector.tensor_copy(rl[DH:DH + 1, :], psO[DH:DH + 1, :])
                lsc = work.tile([P, IC // P], f32, tag="lsc", name="lsc",
                                bufs=3)
                lsr = work.tile([P, IC // P], f32, tag="lsr", name="lsr",
                                bufs=3)
                nc.sync.dma_start(out=lsc[:], in_=rl[DH:DH + 1, :])
                nc.vector.reciprocal(lsr[:], lsc[:])
                nc.sync.dma_start(out=l0[:], in_=lsr[:])
                nc.gpsimd.partition_broadcast(bc[:], l0[:])
                if dbg and half == 0 and cl == 0:
                    nc.sync.dma_start(out=dbc[h * DH:(h + 1) * DH, :],
                                      in_=bc[:])
                if even:
                    nc.vector.tensor_tensor(
                        out=otn[m][0:DH, c * IC:(c + 1) * IC],
                        in0=psO[0:DH, :], in1=bc[:],
                        op=mybir.AluOpType.mult)
                else:
                    otmp = work.tile([DH, IC], bf16, tag="otmp", name="otmp",
                                     bufs=2)
                    nc.vector.tensor_tensor(
                        out=otmp[:], in0=psO[0:DH, :], in1=bc[:],
                        op=mybir.AluOpType.mult)
                    nc.sync.dma_start(
                        out=otn[m][DH:P, c * IC:(c + 1) * IC], in_=otmp[:])

            def emit_attention(half, heads_pairs, filler=None,
                               filler_late=None, pops=1):
                filler = list(filler or [])
                filler_late = list(filler_late or [])
                it = 0
                # software-pipelined emission: the O-matmuls (and the
                # normalize at a chunk's last key tile) trail the S/exp
                # stream by 2 chain-pairs so the PE queue never stalls
                # behind an exp wait. The two heads of a pair live at
                # partition offsets 0/64 of the same kt/qt tile, so their
                # K=64 S-matmuls row-tile into the PE array concurrently.
                tail = []

                def drain(n):
                    while len(tail) > n:
                        tail.pop(0)()

                for m in heads_pairs:
                    psO = {}
                    for hb in range(2):
                        for cl in range(2):
                            psO[hb, cl] = psum.tile(
                                [DH + 1, IC], f32, tag=f"psO{hb}{cl}",
                                name=f"psO{hb}{cl}", bufs=1)
                    njt = 8 * half + 8
                    last_pair = m == heads_pairs[-1]
                    for jt in range(njt):
                        it += 1
                        for _ in range(pops):
                            if filler:
                                filler.pop(0)()
                        cd = jt // 4          # global diagonal chunk
                        cl_lo = max(cd - 2 * half, 0)
                        dcl = cd - 2 * half if cd >= 2 * half else None
                        order = [cl for cl in range(cl_lo, 2) if cl != dcl]
                        if dcl is not None and dcl >= cl_lo:
                            order.append(dcl)
                        for cl in order:
                            c = 2 * half + cl
                            psS = {}
                            for hb in range(2):
                                psS[hb] = psum.tile([P, IC], f32, tag="ps512",
                                                    name="psS", bufs=4)
                            # row-tiled pair: both K=64 matmuls execute
                            # concurrently in the array
                            for hb in range(2):
                                po = hb * DH
                                nc.tensor.matmul(
                                    psS[hb][:],
                                    kt[m][po:po + DH, jt * P:(jt + 1) * P],
                                    qt[m][po:po + DH, c * IC:(c + 1) * IC],
                                    start=True, stop=True)
                            pts = {}
                            for hb in range(2):
                                pt = work.tile([P, IC], bf16, tag="pt",
                                               name="pt", bufs=8)
                                if cl == dcl:
                                    pat = jt - 4 * cd
                                    nc.vector.tensor_tensor(
                                        out=psS[hb][:], in0=psS[hb][:],
                                        in1=cmask[:, pat * IC:(pat + 1) * IC],
                                        op=mybir.AluOpType.add)
                                nc.scalar.activation(
                                    pt[:], psS[hb][:],
                                    mybir.ActivationFunctionType.Exp,
                                    scale=SCALE)
                                pts[hb] = pt

                            def o_mm(m=m, half=half, cl=cl, c=c, jt=jt,
                                     psO_a=psO[0, cl], psO_b=psO[1, cl],
                                     pt_a=pts[0], pt_b=pts[1]):
                                for hb, psO_t, pt_t in ((0, psO_a, pt_a),
                                                        (1, psO_b, pt_b)):
                                    h = 2 * m + hb
                                    nc.tensor.matmul(
                                        psO_t[:],
                                        va[jt][:, h * (DH + 1):
                                               (h + 1) * (DH + 1)],
                                        pt_t[:],
                                        start=(jt == 0),
                                        stop=(jt == 4 * c + 3))
                                    if jt == 4 * c + 3:
                                        emit_normalize(h, half, cl, psO_t)
                            tail.append(o_mm)
                            drain(2)
                        if filler_late and last_pair and jt >= 13:
                            filler_late.pop(0)()
                drain(0)
                for f in filler + filler_late:
                    f()

            def y_unit(tt, ncol):
                def go():
                    psY = psum.tile([P, IC], f32, tag="ps512", name="psY",
                                    bufs=4)
                    for m in range(2):
                        nc.tensor.matmul(
                            psY[:],
                            otn[m][:, tt * P:(tt + 1) * P],
                            wo_t[m][:, ncol * IC:(ncol + 1) * IC],
                            start=(m == 0), stop=(m == 1))
                    ysb = work.tile([P, IC], f16, tag="ysb", name="ysb",
                                    bufs=4)
                    nc.vector.tensor_copy(ysb[:], psY[:])
                    nc.sync.dma_start(
                        out=ybounce[tt * P:(tt + 1) * P,
                                    ncol * IC:(ncol + 1) * IC],
                        in_=ysb[:])
                return go

            def rs_unit(q):
                def go():
                    nc.gpsimd.collective_compute(
                        "ReduceScatter",
                        mybir.AluOpType.add,
                        replica_groups=GROUPS,
                        ins=[ybounce[q * (T // 4):(q + 1) * (T // 4), :]],
                        outs=[rs_out[q][:]],
                    )
                    nc.sync.dma_start(
                        out=out_d[q * (T // 16):(q + 1) * (T // 16), :],
                        in_=rs_out[q][:])
                return go

            def quarter_units(q):
                units = []
                for tt in range(4 * q, 4 * q + 4):
                    for ncol in range(2):
                        units.append(y_unit(tt, ncol))
                units.append(rs_unit(q))
                return units

            # ---- program ----
            with tc.tile_pool(name="qkv_in", bufs=1) as qkv_in:
                xt = []
                wq_t, wk_t, wv_t = [], [], []
                for k in range(KD):
                    x1 = qkv_in.tile([P, T], bf16, name=f"xt{k}")
                    nc.sync.dma_start(out=x1[:], in_=xT_d[k * P:(k + 1) * P, :])
                    xt.append(x1)
                    q1 = qkv_in.tile([P, DHC], bf16, name=f"wqt{k}")
                    k1 = qkv_in.tile([P, DHC], bf16, name=f"wkt{k}")
                    v1 = qkv_in.tile([P, DHC], bf16, name=f"wvt{k}")
                    nc.sync.dma_start(out=q1[:], in_=wq_d[k * P:(k + 1) * P, :])
                    nc.sync.dma_start(out=k1[:], in_=wk_d[k * P:(k + 1) * P, :])
                    nc.sync.dma_start(out=v1[:], in_=wv_d[k * P:(k + 1) * P, :])
                    wq_t.append(q1)
                    wk_t.append(k1)
                    wv_t.append(v1)

                qtkt_unit(0, 0, wq_t, wk_t, xt)()
                qtkt_unit(0, 1, wq_t, wk_t, xt)()
                v_unit(0, wv_t, xt)()
                v_unit(1, wv_t, xt)()
                f0 = ([v_unit(tt, wv_t, xt) for tt in range(2, 8)]
                      + [qtkt_unit(0, ic, wq_t, wk_t, xt) for ic in (2, 3)]
                      + [qtkt_unit(1, ic, wq_t, wk_t, xt)
                         for ic in range(NIC)]
                      + [v_unit(tt, wv_t, xt) for tt in range(8, NT)])
                emit_attention(0, [0], filler=f0, pops=3)
                emit_attention(0, [1])
                if dbg:
                    for m in range(2):
                        nc.sync.dma_start(out=dqt[m * P:(m + 1) * P, :],
                                          in_=qt[m][:])
                        nc.sync.dma_start(out=dkt[m * P:(m + 1) * P, :],
                                          in_=kt[m][:])
                    nc.sync.dma_start(out=dva[:], in_=va[0][:])
            emit_attention(1, [0, 1],
                           filler=quarter_units(0) + quarter_units(1),
                           filler_late=quarter_units(2))
            for f in quarter_units(3):
                f()
            if dbg:
                for m in range(2):
                    nc.sync.dma_start(out=dot[m * P:(m + 1) * P, :],
                                      in_=otn[m][:])

    nc.compile()
    return nc


def make_cmask():
    """4 diagonal-block mask patterns [128, 512], pattern p:
    valid (0.0) iff 128*p + j <= i, else MASKVAL."""
    j = np.arange(P)[:, None]
    i = np.arange(IC)[None, :]
    pats = [np.where(128 * p + j <= i, 0.0, MASKVAL).astype(np.float32)
            for p in range(4)]
    return np.concatenate(pats, axis=1)


def shard_inputs(x, Wq, Wk, Wv, Wo):
    import ml_dtypes
    bf = ml_dtypes.bfloat16
    cmask = make_cmask()
    in_maps = []
    for c in range(N_CORES):
        b, r = divmod(c, 4)
        sl = slice(r * DHC, (r + 1) * DHC)
        in_maps.append({
            "xT": np.ascontiguousarray(x[b].T).astype(bf),
            "wq": np.ascontiguousarray(Wq[:, sl]).astype(bf),
            "wk": np.ascontiguousarray(Wk[:, sl]).astype(bf),
            "wv": np.ascontiguousarray(Wv[:, sl]).astype(bf),
            "wo": np.ascontiguousarray(Wo[sl, :]).astype(bf),
            "cmask": cmask,
            "ones": np.ones((P, H_LOC), dtype=bf),
        })
    return in_maps


def assemble(results, B=2):
    # core (b, r) "out" (f16): rows [q*128:(q+1)*128) = shard of query
    # quarter q; the shard covers rows r*128..(r+1)*128 of that quarter.
    Q = T // 16  # 128
    out = np.empty((B, T, D), dtype=np.float32)
    for c in range(N_CORES):
        b, r = divmod(c, 4)
        res = results[c]["out"].astype(np.float32)
        for q in range(4):
            out[b, q * (T // 4) + r * Q: q * (T // 4) + (r + 1) * Q, :] \
                = res[q * Q:(q + 1) * Q]
    return out


_NC_CACHE = None


def get_nc():
    global _NC_CACHE
    if _NC_CACHE is None:
        _NC_CACHE = build_nc()
    return _NC_CACHE


def run(inputs, trace=False):
    nc = get_nc()
    in_maps = shard_inputs(inputs["x"], inputs["Wq"], inputs["Wk"],
                           inputs["Wv"], inputs["Wo"])
    res = run_bass_kernel_spmd(nc, in_maps, core_ids=list(range(N_CORES)),
                               trace=trace)
    return assemble(res.results), res


def kernel(x, Wq, Wk, Wv, Wo):
    out, _ = run({"x": np.asarray(x), "Wq": np.asarray(Wq),
                  "Wk": np.asarray(Wk), "Wv": np.asarray(Wv),
                  "Wo": np.asarray(Wo)})
    return out
